# revision 1
# baseline (speedup 1.0000x reference)
"""Trainium2 Bass kernel for nn_Attention_47605417509124 — linear-attention
factorization.

Reference per batch b (B=4, N=4096, C=256):
    out = (phi_x @ theta_x.T / N) @ g_x @ W_w.T + W_b + x
with phi_x/theta_x/g_x linear in x and NO softmax, so the N x N score
matrix is rank-C and the product reassociates exactly:
    out = phi_x @ S + resid,   S = theta_x.T @ g2_x   (C x C)
    g2_x = x @ G2 + b2,  G2 = (g_w.T/N) @ W_w.T,  b2 = (g_b/N) @ W_w.T
This collapses the O(N^2 C) score/value matmuls to O(N C^2).

Sharding: 8 cores = 4 batches x 2 halves. Each core computes theta/g2/S
over the full (rotated) sequence and phi/out rows for its own half.
All matmuls in float32r, fp32 PSUM accumulation.
"""

import numpy as np

import concourse.bass as bass
import concourse.mybir as mybir
import concourse.tile as tile
from concourse import bacc
from concourse.bass_utils import run_bass_kernel_spmd

B, N, C = 4, 4096, 256
NCORES = 8
HALF = N // 2
P = 128
JT = N // P          # 32 j tiles
IT = HALF // P       # 16 i tiles

F32 = mybir.dt.float32
F32R = mybir.dt.float32r
AF = mybir.ActivationFunctionType

_CACHE = {}


def _build_module():
    nc = bacc.Bacc("TRN2", target_bir_lowering=False, debug=False,
                   num_devices=NCORES)

    CMB = 3 * C + 1  # PwT | phb | TwT | G2
    xT_d = nc.dram_tensor("xT", [P, 2, N], F32R, kind="ExternalInput")
    comb_d = nc.dram_tensor("comb", [P, 2, CMB], F32R, kind="ExternalInput")
    tbb_d = nc.dram_tensor("tbb", [1, 2, C], F32, kind="ExternalInput")
    gbb_d = nc.dram_tensor("gbb", [1, 2, C], F32, kind="ExternalInput")
    resid_d = nc.dram_tensor("resid", [P, IT, C], F32, kind="ExternalInput")
    out_d = nc.dram_tensor("out", [P, IT, C], F32, kind="ExternalOutput")

    with tile.TileContext(nc) as tc:
        with tc.tile_pool(name="big", bufs=1) as big, \
             tc.tile_pool(name="ps_work", bufs=4, space="PSUM") as psw, \
             tc.tile_pool(name="ps_acc", bufs=4, space="PSUM") as psa:

            xT_sb = big.tile([P, 2, N], F32R)        # rotated x[b].T
            comb_sb = big.tile([P, 2, CMB], F32R)
            phW_sb = comb_sb[:, :, 0:C]
            thW_sb = comb_sb[:, :, C + 1:2 * C + 1]
            gW_sb = comb_sb[:, :, 2 * C + 1:3 * C + 1]

            def phb_col(dh):
                return comb_sb[:, dh, C:C + 1].bitcast(F32)

            tbb_sb = big.tile([P, 2, C], F32)
            gbb_sb = big.tile([P, 2, C], F32)
            phiT_sb = big.tile([P, 2, HALF], F32R)   # phi_x.T (own rows)
            th_sb = big.tile([P, JT, C], F32R)       # theta_x natural
            gx_sb = big.tile([P, JT, C], F32R)       # g2_x natural
            S_sb = big.tile([P, 2, C], F32R)         # S = theta.T @ g2
            resid_sb = big.tile([P, IT, C], F32)     # + output staging

            # ---- input DMAs (consumption order; SP sequencer only) ----
            xT_ap = xT_d.ap()
            CA, CB = C + 1, 2 * C + 1
            nc.sync.dma_start(out=comb_sb[:, :, :CA],
                              in_=comb_d.ap()[:, :, :CA])
            nc.sync.dma_start(out=xT_sb[:, :, 0:256], in_=xT_ap[:, :, 0:256])
            nc.sync.dma_start(out=xT_sb[:, :, 256:512],
                              in_=xT_ap[:, :, 256:512])
            nc.sync.dma_start(out=comb_sb[:, :, CA:CB],
                              in_=comb_d.ap()[:, :, CA:CB])
            nc.sync.dma_start(out=comb_sb[:, :, CB:],
                              in_=comb_d.ap()[:, :, CB:])
            nc.sync.dma_start(out=tbb_sb,
                              in_=tbb_d.ap().to_broadcast([P, 2, C]))
            nc.sync.dma_start(out=gbb_sb,
                              in_=gbb_d.ap().to_broadcast([P, 2, C]))
            for q in range(1, 8):
                nc.sync.dma_start(out=xT_sb[:, :, q * 512:(q + 1) * 512],
                                  in_=xT_ap[:, :, q * 512:(q + 1) * 512])
            nc.sync.dma_start(out=resid_sb[:, :8, :],
                              in_=resid_d.ap()[:, :8, :])
            nc.sync.dma_start(out=resid_sb[:, 8:, :],
                              in_=resid_d.ap()[:, 8:, :])

            # ---- PE warm-up during the initial DMA wait ----
            warm_sb = big.tile([P, 512], F32R)
            warm_dst = big.tile([P, 512], F32)
            nc.vector.memset(warm_sb.bitcast(F32), 0.0)
            ps_warm = psw.tile([P, 512], F32, tag="work", name="ps_warm")
            for wi in range(10):
                nc.tensor.matmul(ps_warm, warm_sb[:, :P], warm_sb,
                                 start=(wi == 0), stop=(wi == 9))
            nc.vector.tensor_copy(out=warm_dst, in_=ps_warm)

            # ---- producers ----
            def prod_phi(kc):
                # phi_x.T [d, i] for own rows; per-partition bias on ACT
                subs = ([slice(0, 256), slice(256, 512)] if kc == 0
                        else [slice(kc * 512, (kc + 1) * 512)])
                for js in subs:
                    for dh in range(2):
                        ps = psw.tile([P, 512], F32, tag="work",
                                      name=f"psph{kc}")
                        w = js.stop - js.start
                        for ch in range(2):
                            nc.tensor.matmul(
                                ps[:, :w],
                                phW_sb[:, ch, dh * P:(dh + 1) * P],
                                xT_sb[:, ch, js],
                                start=(ch == 0), stop=(ch == 1))
                        nc.scalar.activation(
                            out=phiT_sb[:, dh, js], in_=ps[:, :w],
                            func=AF.Identity, bias=phb_col(dh), scale=1.0)

            def prod_nat(kc, w_sb, bias_sb, dst_sb, nm):
                # natural-layout projection [j, d]; two j tiles per bank
                for q2 in range(2):
                    jp = kc * 2 + q2
                    ps = psw.tile([P, 512], F32, tag="work",
                                  name=f"ps{nm}{jp}")
                    for q in range(2):
                        jt = jp * 2 + q
                        for ch in range(2):
                            nc.tensor.matmul(
                                ps[:, q * C:(q + 1) * C],
                                xT_sb[:, ch, jt * P:(jt + 1) * P],
                                w_sb[:, ch, :],
                                start=(ch == 0), stop=(ch == 1))
                    nc.vector.tensor_add(
                        out=dst_sb[:, jp * 2:jp * 2 + 2, :],
                        in0=ps.rearrange("p (t d) -> p t d", d=C),
                        in1=bias_sb)

            psS = [psa.tile([P, C], F32, tag="acc", name=f"psS{q}")
                   for q in range(2)]

            def prod_S(kc):
                # S[c, d] += theta[j, c].T @ g2[j, d] over this chunk's j
                for q in range(4):
                    jt = kc * 4 + q
                    for mh in range(2):
                        nc.tensor.matmul(
                            psS[mh],
                            th_sb[:, jt, mh * P:(mh + 1) * P],
                            gx_sb[:, jt, :],
                            start=(jt == 0), stop=(jt == JT - 1))

            # ---- main chunk loop: projections + S, S two chunks behind
            # (gives the DVE bias-adds lead time over the S matmuls) ----
            for kc in range(8):
                if kc < 4:
                    prod_phi(kc)
                prod_nat(kc, thW_sb, tbb_sb, th_sb, "t")
                prod_nat(kc, gW_sb, gbb_sb, gx_sb, "g")
                if kc >= 2:
                    prod_S(kc - 2)
            prod_S(6)
            prod_S(7)

            # S -> SBUF (both engines in parallel)
            nc.vector.tensor_copy(out=S_sb[:, 0, :], in_=psS[0])
            nc.scalar.copy(out=S_sb[:, 1, :], in_=psS[1])

            # ---- output: out[i, d] = phi[i, :] @ S + resid ----
            # two i tiles per PSUM bank (regions complete sequentially,
            # so the shared-bank start/stop pattern is safe); one DVE
            # add + one store per pair
            out_ap = out_d.ap()
            for itp in range(IT // 2):
                it0 = itp * 2
                ps = psw.tile([P, 512], F32, tag="work", name=f"psy{itp}")
                for q in range(2):
                    it = it0 + q
                    for ch in range(2):
                        nc.tensor.matmul(
                            ps[:, q * C:(q + 1) * C],
                            phiT_sb[:, ch, it * P:(it + 1) * P],
                            S_sb[:, ch, :],
                            start=(ch == 0), stop=(ch == 1))
                nc.vector.tensor_add(
                    out=resid_sb[:, it0:it0 + 2, :],
                    in0=ps.rearrange("p (t d) -> p t d", d=C),
                    in1=resid_sb[:, it0:it0 + 2, :])
                nc.sync.dma_start(out=out_ap[:, it0:it0 + 2, :],
                                  in_=resid_sb[:, it0:it0 + 2, :])

    nc.finalize()
    return nc


def _get_module():
    if "nc" not in _CACHE:
        _CACHE["nc"] = _build_module()
    return _CACHE["nc"]


def _to_sbuf_layout(a):
    o = a.shape[0] // P
    return np.ascontiguousarray(a.reshape(o, P, *a.shape[1:]).swapaxes(0, 1))


def _prep_in_maps(x, g_w, g_b, theta_w, theta_b, phi_w, phi_b, W_w, W_b):
    x = np.ascontiguousarray(np.asarray(x, dtype=np.float32))
    f32 = np.float32

    phW = _to_sbuf_layout(np.ascontiguousarray(np.asarray(phi_w, f32).T))
    thW = _to_sbuf_layout(np.ascontiguousarray(np.asarray(theta_w, f32).T))
    G2 = (np.asarray(g_w, np.float64).T / N) @ np.asarray(W_w, np.float64).T
    b2 = (np.asarray(g_b, np.float64) / N) @ np.asarray(W_w, np.float64).T
    gW = _to_sbuf_layout(np.ascontiguousarray(G2.astype(f32)))
    phb = np.ascontiguousarray(
        np.asarray(phi_b, f32).reshape(2, P).T)[:, :, None]
    comb = np.ascontiguousarray(
        np.concatenate([phW, phb, thW, gW], axis=2))
    tbb = np.ascontiguousarray(
        np.broadcast_to(np.asarray(theta_b, f32), (1, 2, C)))
    gbb = np.ascontiguousarray(np.broadcast_to(b2.astype(f32), (1, 2, C)))
    W_b = np.asarray(W_b, f32)

    in_maps = []
    for core in range(NCORES):
        b, h = core // 2, core % 2
        rows = slice(h * HALF, (h + 1) * HALF)
        other = slice(0, HALF) if h else slice(HALF, N)
        xb = x[b]
        xrot_T = np.concatenate([xb[rows], xb[other]], axis=0).T
        resid = xb[rows] + W_b
        in_maps.append({
            "xT": _to_sbuf_layout(np.ascontiguousarray(xrot_T)),
            "comb": comb, "tbb": tbb, "gbb": gbb,
            "resid": _to_sbuf_layout(resid),
        })
    return in_maps


def _get_runner():
    if "runner" in _CACHE:
        return _CACHE["runner"]
    import jax
    from jax.sharding import Mesh, PartitionSpec
    try:
        from jax.experimental.shard_map import shard_map
    except Exception:
        from jax.shard_map import shard_map
    from concourse import bass2jax, mybir as mb

    nc = _get_module()
    bass2jax.install_neuronx_cc_hook()
    partition_name = (nc.partition_id_tensor.name
                      if nc.partition_id_tensor else None)

    in_names, out_names, out_avals, zero_shapes = [], [], [], []
    for alloc in nc.m.functions[0].allocations:
        if not isinstance(alloc, mb.MemoryLocationSet):
            continue
        name = alloc.memorylocations[0].name
        if alloc.kind == "ExternalInput":
            if name != partition_name:
                in_names.append(name)
        elif alloc.kind == "ExternalOutput":
            shape = tuple(alloc.tensor_shape)
            dtype = mb.dt.np(alloc.dtype)
            out_names.append(name)
            out_avals.append(jax.core.ShapedArray(shape, dtype))
            zero_shapes.append((shape, dtype))
    n_params = len(in_names)
    all_names = in_names + out_names
    if partition_name is not None:
        all_names.append(partition_name)
    donate = tuple(range(n_params, n_params + len(out_names)))

    def _body(*args):
        operands = list(args)
        if partition_name is not None:
            operands.append(bass2jax.partition_id_tensor())
        outs = bass2jax._bass_exec_p.bind(
            *operands,
            out_avals=tuple(out_avals),
            in_names=tuple(all_names),
            out_names=tuple(out_names),
            lowering_input_output_aliases=(),
            sim_require_finite=True,
            sim_require_nnan=True,
            nc=nc,
        )
        return tuple(outs)

    try:
        devices = jax.devices("axon")[:NCORES]
    except Exception:
        devices = jax.devices()[:NCORES]
    mesh = Mesh(np.asarray(devices), ("core",))
    nin = n_params + len(out_names)
    sharded = jax.jit(
        shard_map(_body, mesh=mesh,
                  in_specs=(PartitionSpec("core"),) * nin,
                  out_specs=(PartitionSpec("core"),) * len(out_names),
                  check_rep=False),
        donate_argnums=donate, keep_unused=True)

    def run(in_maps):
        concat_in = [
            np.concatenate([np.asarray(in_maps[c][nm])
                            for c in range(NCORES)], axis=0)
            for nm in in_names]
        concat_zeros = [np.zeros((NCORES * s[0], *s[1:]), dt)
                        for s, dt in zero_shapes]
        out_arrs = sharded(*concat_in, *concat_zeros)
        return [
            {nm: np.asarray(out_arrs[i]).reshape(
                NCORES, *zero_shapes[i][0])[c]
             for i, nm in enumerate(out_names)}
            for c in range(NCORES)]

    _CACHE["runner"] = run
    return run


def kernel(x, g_w, g_b, theta_w, theta_b, phi_w, phi_b, W_w, W_b):
    in_maps = _prep_in_maps(x, g_w, g_b, theta_w, theta_b, phi_w, phi_b,
                            W_w, W_b)
    try:
        results = _get_runner()(in_maps)
    except Exception:
        _CACHE.pop("runner", None)
        nc = _get_module()
        results = run_bass_kernel_spmd(
            nc, in_maps, core_ids=list(range(NCORES))).results
    out = np.empty((B, N, C), dtype=np.float32)
    for core in range(NCORES):
        b, h = core // 2, core % 2
        o = results[core]["out"]
        out[b, h * HALF:(h + 1) * HALF, :] = (
            o.swapaxes(0, 1).reshape(HALF, C))
    return out



# revision 5
# speedup vs baseline: 1.5935x; 1.5935x over previous
"""Trainium2 Bass kernel for nn_Attention_47605417509124 — Gram-matrix
factorization of softmax-free attention.

Reference per batch b (B=4, N=4096, C=256):
    out = (phi_x @ theta_x.T / N) @ g_x @ W_w.T + W_b + x
Everything is linear, so with x~ = [x | 1] (N x 257), T~ = [theta_w.T;
theta_b], G~ = [g_w.T W_w.T / N; g_b W_w.T / N], H = T~ @ phi_w,
hp = T~ @ phi_b (all host-folded):
    M~   = x~.T @ x~                 (257 x 257 Gram, one N-contraction)
    W    = M~ @ G~                   (257 x 256)
    A    = H.T @ W + I               (256 x 256)
    crow = W.T[d,:] @ hp + W_b       (256 col)
    out  = x @ A + crow              (the only other N-sized matmul)
This needs just 2*N*C^2 MACs per batch vs 6*N*C^2 for the projection
formulation, plus a tiny C^3 chain.

Sharding: 8 cores = 4 batches x 2 halves. Each core computes the Gram
chain over the full batch (duplicated in the pair) and output rows for
its half, produced transposed (outT[d,i]) so crow is a per-partition
ACT/DVE bias during the PSUM->SBUF pass. x / xT / A / out in fp16
(1 cycle/row on PE, half DMA); chain in f32r with fp32 PSUM.
"""

import numpy as np

import concourse.bass as bass
import concourse.mybir as mybir
import concourse.tile as tile
from concourse import bacc
from concourse.bass_utils import run_bass_kernel_spmd

B, N, C = 4, 4096, 256
NCORES = 8
HALF = N // 2
P = 128
JT = N // P          # 32 j tiles
CA = C + 1           # 257 augmented columns

F32 = mybir.dt.float32
F32R = mybir.dt.float32r
F16 = mybir.dt.float16
AF = mybir.ActivationFunctionType

_CACHE = {}


def _build_module():
    nc = bacc.Bacc("TRN2", target_bir_lowering=False, debug=False,
                   num_devices=NCORES)

    xa_d = nc.dram_tensor("xa", [P, JT, CA], F16, kind="ExternalInput")
    xT_d = nc.dram_tensor("xT", [P, 2, HALF], F16, kind="ExternalInput")
    gt_d = nc.dram_tensor("gt", [P, 2, C], F32R, kind="ExternalInput")
    hm_d = nc.dram_tensor("hm", [P, 2, C], F32R, kind="ExternalInput")
    idn_d = nc.dram_tensor("idn", [P, 2, C], F32R, kind="ExternalInput")
    cols_d = nc.dram_tensor("cols", [P, 2, 2], F32R, kind="ExternalInput")
    rows_d = nc.dram_tensor("rows", [1, 514], F32R, kind="ExternalInput")
    ot_d = nc.dram_tensor("ot", [P, 2, HALF], F16, kind="ExternalOutput")

    with tile.TileContext(nc) as tc:
        with tc.tile_pool(name="big", bufs=1) as big, \
             tc.tile_pool(name="ps_acc", bufs=2, space="PSUM") as psa, \
             tc.tile_pool(name="ps_work", bufs=4, space="PSUM") as psw, \
             tc.tile_pool(name="ps_out", bufs=2, space="PSUM") as pso:

            xa_sb = big.tile([P, JT, CA], F16)
            xT_sb = big.tile([P, 2, HALF], F16)
            gt_sb = big.tile([P, 2, C], F32R)
            hm_sb = big.tile([P, 2, C], F32R)
            idn_sb = big.tile([P, 2, C], F32R)
            cols_sb = big.tile([P, 2, 2], F32R)
            rows_sb = big.tile([1, 514], F32R)
            Msb = big.tile([P, 2, CA], F32R)
            srow_sb = big.tile([1, C], F32R)
            Wsb = big.tile([P, 2, C], F32R)
            Wrow_sb = big.tile([1, C], F32R)
            A_sb = big.tile([P, 2, C], F16)
            crow_sb = big.tile([P, 2, 1], F32)
            out_sb = big.tile([P, 2, HALF], F16)

            # ---- input DMAs in consumption order (SP sequencer) ----
            xa_ap, xT_ap = xa_d.ap(), xT_d.ap()
            chunks = [(0, 2), (2, 2)] + [(4 * k, 4) for k in range(1, 8)]
            j0, nj = chunks[0]
            nc.sync.dma_start(out=xa_sb[:, j0:j0 + nj, :],
                              in_=xa_ap[:, j0:j0 + nj, :])
            nc.sync.dma_start(out=idn_sb, in_=idn_d.ap())
            nc.sync.dma_start(out=gt_sb, in_=gt_d.ap())
            nc.sync.dma_start(out=rows_sb, in_=rows_d.ap())
            for j0, nj in chunks[1:]:
                nc.sync.dma_start(out=xa_sb[:, j0:j0 + nj, :],
                                  in_=xa_ap[:, j0:j0 + nj, :])
            nc.sync.dma_start(out=hm_sb, in_=hm_d.ap())
            nc.sync.dma_start(out=cols_sb, in_=cols_d.ap())
            for q in range(4):
                nc.sync.dma_start(out=xT_sb[:, :, q * 512:(q + 1) * 512],
                                  in_=xT_ap[:, :, q * 512:(q + 1) * 512])

            # ---- PE warm-up during the first DMA wait ----
            warm_sb = big.tile([P, 384], F32R)
            warm_dst = big.tile([P, 384], F32)
            nc.vector.memset(warm_sb.bitcast(F32), 0.0)
            ps_warm = psw.tile([P, 384], F32, tag="work", name="ps_warm")
            nc.tensor.matmul(ps_warm, warm_sb[:, :P], warm_sb,
                             start=True, stop=True)
            nc.vector.tensor_copy(out=warm_dst, in_=ps_warm)

            # ---- Gram matrix M~ = x~.T @ x~, fp16 in, fp32 accum ----
            psM = [psa.tile([P, CA], F32, tag="acc", name=f"psM{mh}")
                   for mh in range(2)]
            for jt in range(JT):
                for mh in range(2):
                    nc.tensor.matmul(
                        psM[mh],
                        xa_sb[:, jt, mh * P:(mh + 1) * P],
                        xa_sb[:, jt, :],
                        start=(jt == 0), stop=(jt == JT - 1))
            nc.vector.tensor_copy(out=Msb[:, 0, :], in_=psM[0])
            nc.scalar.copy(out=Msb[:, 1, :], in_=psM[1])

            # ---- s row: s[cc] = sum_j x~[j, cc], via s-col x identity ----
            ps_sr = psw.tile([1, C], F32, tag="work", name="ps_sr")
            for mh in range(2):
                nc.tensor.matmul(ps_sr, Msb[:, mh, C:CA], idn_sb[:, mh, :],
                                 start=(mh == 0), stop=(mh == 1))
            nc.vector.tensor_copy(out=srow_sb, in_=ps_sr)

            # ---- W = M~ @ G~  (rows 0:256 in two chunks + row 256) ----
            b2_row = rows_sb[0:1, 0:C]
            h_row = rows_sb[0:1, C:2 * C]
            hp256 = rows_sb[0:1, 2 * C:2 * C + 1]
            nval = rows_sb[0:1, 2 * C + 1:2 * C + 2]
            psW = [psw.tile([P, C], F32, tag="work", name=f"psW{k}")
                   for k in range(2)]
            for k in range(2):
                sl = slice(k * P, (k + 1) * P)
                nc.tensor.matmul(psW[k], Msb[:, 0, sl], gt_sb[:, 0, :],
                                 start=True, stop=False)
                nc.tensor.matmul(psW[k], Msb[:, 1, sl], gt_sb[:, 1, :],
                                 start=False, stop=False)
                nc.tensor.matmul(psW[k], srow_sb[0:1, sl], b2_row,
                                 start=False, stop=True)
            psWr = psw.tile([1, C], F32, tag="work", name="psWr")
            nc.tensor.matmul(psWr, Msb[:, 0, C:CA], gt_sb[:, 0, :],
                             start=True, stop=False)
            nc.tensor.matmul(psWr, Msb[:, 1, C:CA], gt_sb[:, 1, :],
                             start=False, stop=False)
            nc.tensor.matmul(psWr, nval, b2_row, start=False, stop=True)
            nc.vector.tensor_copy(out=Wsb[:, 0, :], in_=psW[0])
            nc.scalar.copy(out=Wsb[:, 1, :], in_=psW[1])
            nc.vector.tensor_copy(out=Wrow_sb, in_=psWr)

            # ---- A = H.T @ W + I ;  crow = W.T @ hp + W_b (column) ----
            psA = [psw.tile([P, C], F32, tag="work", name=f"psA{k}")
                   for k in range(2)]
            psC = [psw.tile([P, 2], F32, tag="work", name=f"psC{k}")
                   for k in range(2)]
            hp_col = cols_sb[:, :, 0:1]
            wb_col = cols_sb[:, :, 1:2]
            for k in range(2):
                sl = slice(k * P, (k + 1) * P)
                nc.tensor.matmul(psA[k], hm_sb[:, 0, sl], Wsb[:, 0, :],
                                 start=True, stop=False)
                nc.tensor.matmul(psA[k], hm_sb[:, 1, sl], Wsb[:, 1, :],
                                 start=False, stop=False)
                nc.tensor.matmul(psA[k], h_row[:, sl], Wrow_sb,
                                 start=False, stop=True)
                nc.tensor.matmul(psC[k], Wsb[:, 0, sl], cols_sb[:, 0, :],
                                 start=True, stop=False)
                nc.tensor.matmul(psC[k], Wsb[:, 1, sl], cols_sb[:, 1, :],
                                 start=False, stop=False)
                nc.tensor.matmul(psC[k], Wrow_sb[0:1, sl],
                                 rows_sb[0:1, 2 * C:2 * C + 2],
                                 start=False, stop=True)
                nc.vector.tensor_add(out=A_sb[:, k, :], in0=psA[k],
                                     in1=idn_sb[:, k, :].bitcast(F32))
                nc.vector.tensor_add(out=crow_sb[:, k, :],
                                     in0=psC[k][:, 0:1],
                                     in1=wb_col[:, k, :].bitcast(F32))

            # ---- outT[d, i] = sum_c A[c, d] xT[c, i]  (+ crow bias) ----
            ot_ap = ot_d.ap()
            for dh in range(2):
                dsl = slice(dh * P, (dh + 1) * P)
                for q in range(4):
                    isl = slice(q * 512, (q + 1) * 512)
                    ps = pso.tile([P, 512], F32, tag="out", name=f"psO{q}")
                    nc.tensor.matmul(ps, A_sb[:, 0, dsl], xT_sb[:, 0, isl],
                                     start=True, stop=False)
                    nc.tensor.matmul(ps, A_sb[:, 1, dsl], xT_sb[:, 1, isl],
                                     start=False, stop=True)
                    if q % 2 == 0:
                        nc.scalar.activation(
                            out=out_sb[:, dh, isl], in_=ps,
                            func=AF.Identity, bias=crow_sb[:, dh, :],
                            scale=1.0)
                    else:
                        nc.vector.tensor_scalar_add(
                            out=out_sb[:, dh, isl], in0=ps,
                            scalar1=crow_sb[:, dh, :])
                    nc.sync.dma_start(out=ot_ap[:, dh, isl],
                                      in_=out_sb[:, dh, isl])

    nc.finalize()
    return nc


def _get_module():
    if "nc" not in _CACHE:
        _CACHE["nc"] = _build_module()
    return _CACHE["nc"]


def _to_sbuf_layout(a):
    o = a.shape[0] // P
    return np.ascontiguousarray(a.reshape(o, P, *a.shape[1:]).swapaxes(0, 1))


def _prep_in_maps(x, g_w, g_b, theta_w, theta_b, phi_w, phi_b, W_w, W_b):
    f32, f64, f16 = np.float32, np.float64, np.float16
    x = np.asarray(x, f32)

    G2 = (np.asarray(g_w, f64).T / N) @ np.asarray(W_w, f64).T
    b2 = (np.asarray(g_b, f64) / N) @ np.asarray(W_w, f64).T
    Ttil = np.concatenate([np.asarray(theta_w, f64).T,
                           np.asarray(theta_b, f64)[None]], 0)
    Gtil = np.concatenate([G2, b2[None]], 0)
    H = Ttil @ np.asarray(phi_w, f64)
    hp = Ttil @ np.asarray(phi_b, f64)

    gt = _to_sbuf_layout(Gtil[0:C].astype(f32))
    hm = _to_sbuf_layout(H[0:C].astype(f32))
    idn = _to_sbuf_layout(np.eye(C, dtype=f32))
    cols = np.ascontiguousarray(np.stack(
        [hp[0:C].astype(f32).reshape(2, P).T,
         np.asarray(W_b, f32).reshape(2, P).T], axis=2))
    rows = np.concatenate(
        [Gtil[C], H[C], [hp[C]], [float(N)]]).astype(f32)[None]
    rows = np.ascontiguousarray(rows)

    in_maps = []
    ones = np.ones((N, 1), f16)
    for core in range(NCORES):
        b, h = core // 2, core % 2
        xb = x[b]
        xa = np.concatenate([xb.astype(f16), ones], axis=1)
        xa = np.ascontiguousarray(
            xa.reshape(JT, P, CA).swapaxes(0, 1))
        xT = np.ascontiguousarray(
            xb[h * HALF:(h + 1) * HALF].T.astype(f16)
            .reshape(2, P, HALF).swapaxes(0, 1))
        in_maps.append({
            "xa": xa, "xT": xT, "gt": gt, "hm": hm, "idn": idn,
            "cols": cols, "rows": rows,
        })
    return in_maps


def _get_runner():
    if "runner" in _CACHE:
        return _CACHE["runner"]
    import jax
    from jax.sharding import Mesh, PartitionSpec
    try:
        from jax.experimental.shard_map import shard_map
    except Exception:
        from jax.shard_map import shard_map
    from concourse import bass2jax, mybir as mb

    nc = _get_module()
    bass2jax.install_neuronx_cc_hook()
    partition_name = (nc.partition_id_tensor.name
                      if nc.partition_id_tensor else None)

    in_names, out_names, out_avals, zero_shapes = [], [], [], []
    for alloc in nc.m.functions[0].allocations:
        if not isinstance(alloc, mb.MemoryLocationSet):
            continue
        name = alloc.memorylocations[0].name
        if alloc.kind == "ExternalInput":
            if name != partition_name:
                in_names.append(name)
        elif alloc.kind == "ExternalOutput":
            shape = tuple(alloc.tensor_shape)
            dtype = mb.dt.np(alloc.dtype)
            out_names.append(name)
            out_avals.append(jax.core.ShapedArray(shape, dtype))
            zero_shapes.append((shape, dtype))
    n_params = len(in_names)
    all_names = in_names + out_names
    if partition_name is not None:
        all_names.append(partition_name)
    donate = tuple(range(n_params, n_params + len(out_names)))

    def _body(*args):
        operands = list(args)
        if partition_name is not None:
            operands.append(bass2jax.partition_id_tensor())
        outs = bass2jax._bass_exec_p.bind(
            *operands,
            out_avals=tuple(out_avals),
            in_names=tuple(all_names),
            out_names=tuple(out_names),
            lowering_input_output_aliases=(),
            sim_require_finite=True,
            sim_require_nnan=True,
            nc=nc,
        )
        return tuple(outs)

    try:
        devices = jax.devices("axon")[:NCORES]
    except Exception:
        devices = jax.devices()[:NCORES]
    mesh = Mesh(np.asarray(devices), ("core",))
    nin = n_params + len(out_names)
    sharded = jax.jit(
        shard_map(_body, mesh=mesh,
                  in_specs=(PartitionSpec("core"),) * nin,
                  out_specs=(PartitionSpec("core"),) * len(out_names),
                  check_rep=False),
        donate_argnums=donate, keep_unused=True)

    def run(in_maps):
        concat_in = [
            np.concatenate([np.asarray(in_maps[c][nm])
                            for c in range(NCORES)], axis=0)
            for nm in in_names]
        concat_zeros = [np.zeros((NCORES * s[0], *s[1:]), dt)
                        for s, dt in zero_shapes]
        out_arrs = sharded(*concat_in, *concat_zeros)
        return [
            {nm: np.asarray(out_arrs[i]).reshape(
                NCORES, *zero_shapes[i][0])[c]
             for i, nm in enumerate(out_names)}
            for c in range(NCORES)]

    _CACHE["runner"] = run
    return run


def kernel(x, g_w, g_b, theta_w, theta_b, phi_w, phi_b, W_w, W_b):
    in_maps = _prep_in_maps(x, g_w, g_b, theta_w, theta_b, phi_w, phi_b,
                            W_w, W_b)
    try:
        results = _get_runner()(in_maps)
    except Exception:
        _CACHE.pop("runner", None)
        nc = _get_module()
        results = run_bass_kernel_spmd(
            nc, in_maps, core_ids=list(range(NCORES))).results
    out = np.empty((B, N, C), dtype=np.float32)
    for core in range(NCORES):
        b, h = core // 2, core % 2
        o = results[core]["ot"]          # [P, 2, HALF] f16, outT layout
        out[b, h * HALF:(h + 1) * HALF, :] = (
            o.transpose(2, 1, 0).reshape(HALF, C).astype(np.float32))
    return out
